# revision 42
# baseline (speedup 1.0000x reference)
"""Multi-head self-attention on 8 Trainium2 NeuronCores (Bass/Tile).

Problem: x[2,2048,1024] -> MHA(16 heads, d_head 64) -> out[2,2048,1024].

Sharding (batch x head-group, Megatron-ish, collective-free):
  core c (0..7): batch b = c//4, head group g = c%4 (heads 4g..4g+3).
  Each core computes q/k/v projections for its 4 heads over its batch,
  attention for those heads, and a PARTIAL output projection
  attn_local[256ch] @ w_out[256ch rows] over the full sequence. The host
  sums the 4 partials per batch (the Megatron row-parallel all-reduce is
  folded into the unshard step; b_out and the V-bias term bv @ w_out are
  added once on the host -- exact, since softmax rows sum to 1).

On-core layout (TensorE compute in bf16, fp32 PSUM accumulation):
  - ACT (exp for softmax) is the bottleneck engine: 16.8M exps/core ~=
    147us of ACT instruction time. The schedule saturates ACT from the
    earliest possible point after the ~7us engine-boot preamble:
    * all inputs arrive in host-pre-blocked layouts that are contiguous
      per SBUF partition (cheap HWDGE triggers, 4-8KB DMA lines), split
      across the sync and scalar queues with x^T token-sliced;
    * k/q chunk-0 projections are emitted first so scores round 0 feeds
      ACT immediately; V projections and the remaining q/k chunks are
      pure PE filler behind ACT pacing.
  - exp p-tiles are paired per round ([128, 2heads, 8kp, 1024]) with 3
    rotating buffers = 3 rounds in flight, so round r+2's exps never
    wait on round r's PV consumption.
  - qT/kT in [channel, t] layout: scores^T = kT.T @ qT with the two
    heads of a chunk in partitions 0-63/64-127 -> concurrent K=64
    matmuls in disjoint PE row groups.
  - softmax: scores^T [128ki, qi] -> ACT exp (PSUM->SBUF bf16,
    scale=1/8 folded, no max subtraction: |s|/8 <= ~2).
  - PV: attn^T = V.T @ P~ as column-tiled concurrent M=64 head pairs;
    denominators via DVE bf16 add-tree + K=128 ones-matmul fold,
    reciprocal_approx_fast straight off PSUM; normalize one round
    behind PV (rep-matmul broadcast + DVE mul); out-projection +
    output DMA per query group.
"""

import numpy as np
import ml_dtypes

import concourse.bass as bass
import concourse.mybir as mybir
import concourse.tile as tile
from concourse import bacc
from concourse import bass_utils
from concourse.bass import ts

BF = mybir.dt.bfloat16
F32 = mybir.dt.float32

B, T, C = 2, 2048, 1024
H, DH = 16, 64
N_CORES = 8
HG = 4  # heads per core
CH = HG * DH  # 256 channels per core

LAST_RESULT = None  # BassKernelResults of the most recent run (for profiling)
_NC_CACHE = None


def _build_nc():
    nc = bacc.Bacc(
        "TRN2", target_bir_lowering=False, debug=False, num_devices=N_CORES
    )

    # host-pre-blocked layouts: every tensor is contiguous along its SBUF
    # partition's free dim, so each DMA is 128 x (one fat line).
    F8 = mybir.dt.float8e4
    xt = nc.dram_tensor("xt", [128, 4, 8, 512], BF, kind="ExternalInput")
    xq8 = nc.dram_tensor("xq8", [128, 4, 4, 2, 512], F8, kind="ExternalInput")
    wq = nc.dram_tensor("wq", [128, 4, 2, CH], F8, kind="ExternalInput")
    wk = nc.dram_tensor("wk", [128, 4, 2, CH], F8, kind="ExternalInput")
    wv = nc.dram_tensor("wv", [128, 8, CH], BF, kind="ExternalInput")
    bqt = nc.dram_tensor("bqt", [128, 2], F32, kind="ExternalInput")
    bkt = nc.dram_tensor("bkt", [128, 2], F32, kind="ExternalInput")
    wout = nc.dram_tensor("wout", [128, 2, C], BF, kind="ExternalInput")
    out = nc.dram_tensor("out", [T, C], BF, kind="ExternalOutput")

    with tile.TileContext(nc) as tc:
        with (
            tc.tile_pool(name="persist", bufs=1) as persist,
            tc.tile_pool(name="consts", bufs=1) as consts,
            tc.tile_pool(name="sbn", bufs=2) as sbn,
            tc.tile_pool(name="osb", bufs=3) as osb,
            tc.tile_pool(name="ps_st", bufs=2, space="PSUM") as ps_st,
            tc.tile_pool(name="ps_pv", bufs=1, space="PSUM") as ps_pv,
            tc.tile_pool(name="ps_misc", bufs=3, space="PSUM") as ps_misc,
        ):
            ones_bf = consts.tile([1, 128], BF)
            nc.vector.memset(ones_bf[:], 1.0)
            ones_col = consts.tile([128, 1], BF)
            nc.vector.memset(ones_col[:], 1.0)


            # fp8 x (DoubleRow-packed) for q/k; bf16 x arrives per-tt into
            # a rotating buffer for the V projections.
            x8_sb = persist.tile([128, 4, 4, 2, 512], F8, tag="x8")
            wq_sb = persist.tile([128, 4, 2, CH], F8, tag="wq")
            wk_sb = persist.tile([128, 4, 2, CH], F8, tag="wk")
            wv_sb = persist.tile([128, 8, CH], BF, tag="wv")
            wout_sb = persist.tile([128, 2, C], BF, tag="wout")
            bqt_sb = consts.tile([128, 2], F32)
            bkt_sb = consts.tile([128, 2], F32)

            # ---- input DMA: 3-way queue split, critical-first ----
            # per-queue bandwidth is only ~65-160GB/s, so the x8 token
            # blocks are spread over all three queues (tt0+tt1 sync,
            # tt2 scalar, tt3 gpsimd): all of x8 lands by ~6us instead
            # of ~14us, so the kT chunk projections never starve the
            # score/exp stream. Bulk V-path loads queue FIFO behind the
            # critical loads on their queue (self-gating).
            nc.sync.dma_start(out=x8_sb[:, 0], in_=xq8[:, 0])
            nc.scalar.dma_start(out=bkt_sb[:], in_=bkt[:])
            nc.scalar.dma_start(out=wk_sb[:], in_=wk[:])
            nc.gpsimd.dma_start(out=bqt_sb[:], in_=bqt[:])
            nc.gpsimd.dma_start(out=wq_sb[:], in_=wq[:])
            nc.sync.dma_start(out=x8_sb[:, 1], in_=xq8[:, 1])
            nc.scalar.dma_start(out=x8_sb[:, 2], in_=xq8[:, 2])
            nc.gpsimd.dma_start(out=x8_sb[:, 3], in_=xq8[:, 3])

            warm_src = consts.tile([128, 512], BF)
            nc.vector.memset(warm_src[:], 0.0)

            # ---- persistent activations ----
            # qkT[:, 0:2, :] = qT chunks (hp), [:, 2:4, :] = kT chunks;
            # chunk hp rows 0-63 = head 2hp, rows 64-127 = head 2hp+1.
            qkT = persist.tile([128, 4, T], BF, tag="qkT")
            vext = persist.tile([128, T // 128, HG, DH], BF, tag="vext")
            attn_p = [
                [
                    persist.tile(
                        [128, 512], BF, tag=f"attnp{hp}_{qg}",
                        name=f"attnp{hp}_{qg}",
                    )
                    for qg in range(4)
                ]
                for hp in range(2)
            ]

            def qk_group(w_i, co, tt):
                """one [128,512] tile of qT (w_i=0) or kT (w_i=1), chunk co"""
                wsb = wq_sb if w_i == 0 else wk_sb
                bias_sb = bqt_sb if w_i == 0 else bkt_sb
                qp = ps_misc.tile([128, 512], F32, tag="sm", name="qp")
                for jp in range(4):
                    nc.tensor.matmul(
                        qp[:],
                        wsb[:, jp, :, ts(co, 128)],
                        x8_sb[:, tt, jp, :, :],
                        perf_mode=mybir.MatmulPerfMode.DoubleRow,
                        start=(jp == 0),
                        stop=(jp == 3),
                    )
                # bias-add + cast on the DVE (keeps the ACT queue for exps)
                nc.vector.tensor_scalar_add(
                    qkT[:, 2 * w_i + co, ts(tt, 512)],
                    qp[:],
                    bias_sb[:, co : co + 1],
                )

            def v_group(tv):
                vp = ps_misc.tile([128, CH], F32, tag="sm", name="vp")
                for ci in range(8):
                    nc.tensor.matmul(
                        vp[:],
                        xv_tiles[tv // 4][:, ci, ts(tv % 4, 128)],
                        wv_sb[:, ci, :],
                        start=(ci == 0),
                        stop=(ci == 7),
                    )
                nc.vector.tensor_copy(
                    vext[:, tv, :, :],
                    vp[:].rearrange("p (h d) -> p h d", h=HG),
                )

            p_tiles = {}
            rec_tiles = {}
            tmp_tiles = {}

            def p_alloc(qg, hp):
                # paired tile: [p, head(A/B), kp, 1024]; one pool slot per
                # round -> bufs=3 keeps 3 rounds of exps live.
                p = osb.tile([128, 2, 8, 1024], BF, tag="p", bufs=3, name="p")
                p_tiles[(qg, hp)] = p
                return p

            def st_seg(qg, hp, kis, p):
                """scores^T + exp for head pair hp, query group qg, ki range.

                One PSUM tile holds BOTH heads' scores for a 128-key block
                (A in cols 0:512, B in 512:1024): the two K=64 matmuls hit
                disjoint PE row groups (0/64) and are adjacent in issue
                order, so they execute concurrently (row tiling)."""
                qs = ts(qg, 512)
                for ki in kis:
                    stp = ps_st.tile([128, 1024], F32, tag="st", name="st")
                    nc.tensor.matmul(
                        stp[:, 0:512],
                        qkT[0:64, 2 + hp, ts(ki, 128)],
                        qkT[0:64, hp, qs],
                        start=True, stop=True,
                    )
                    nc.tensor.matmul(
                        stp[:, 512:1024],
                        qkT[64:128, 2 + hp, ts(ki, 128)],
                        qkT[64:128, hp, qs],
                        start=True, stop=True,
                    )
                    # exp out: head A -> p[:, 0, ki//2, (ki%2)*512 :],
                    # head B -> p[:, 1, ...] (strided 3D AP, one ACTIVATE)
                    nc.scalar.activation(
                        p[:, 0:2, ki // 2, ts(ki % 2, 512)], stp[:],
                        mybir.ActivationFunctionType.Exp, scale=1.0 / 8.0,
                    )

            def st_part(qg, hp):
                p = p_alloc(qg, hp)
                st_seg(qg, hp, range(16), p)

            def pv_part(qg, hp):
                p = p_tiles.pop((qg, hp))
                t4s = {}
                for hh in range(2):
                    # incremental 4-way tree: s_i = kp(2i)+kp(2i+1) can run
                    # as soon as those exps land, so the post-last-exp
                    # critical chain is only s3 -> u1 -> v -> t4 (~2us)
                    # instead of a full 4-level tree (~4.4us).
                    s = []
                    for i in range(4):
                        si = sbn.tile(
                            [128, 1024], BF, tag=f"s{i}", name=f"s{i}", bufs=1
                        )
                        nc.vector.tensor_add(
                            si[:], p[:, hh, 2 * i, :], p[:, hh, 2 * i + 1, :]
                        )
                        s.append(si)
                    u0 = sbn.tile([128, 1024], BF, tag="u0", name="u0", bufs=1)
                    nc.vector.tensor_add(u0[:], s[0][:], s[1][:])
                    u1 = sbn.tile([128, 1024], BF, tag="u1", name="u1", bufs=1)
                    nc.vector.tensor_add(u1[:], s[2][:], s[3][:])
                    v3 = sbn.tile([128, 1024], BF, tag="v3", name="v3", bufs=1)
                    nc.vector.tensor_add(v3[:], u0[:], u1[:])
                    t4 = sbn.tile([128, 512], BF, tag="t4", name="t4", bufs=2)
                    nc.vector.tensor_add(
                        t4[:], v3[:, 0:512], v3[:, 512:1024]
                    )
                    t4s[hh] = t4
                pv = ps_pv.tile([128, 512], F32, tag="pv", name="pv")
                for ki in range(16):
                    for hh in range(2):
                        h = 2 * hp + hh
                        nc.tensor.matmul(
                            pv[64 * hh : 64 * hh + 64, :],
                            vext[:, ki, h, :],
                            p[:, hh, ki // 2, ts(ki % 2, 512)],
                            start=(ki == 0),
                            stop=(ki == 15),
                        )
                # both heads' reciprocals free-packed in one [1,2,512]
                # tile so the bf16 cast runs once per part.
                rec32 = sbn.tile([1, 2, 512], F32, tag="rec32", name="rc", bufs=1)
                for hh in range(2):
                    dps = ps_misc.tile([128, 512], F32, tag="sm", name="dps")
                    nc.tensor.matmul(
                        dps[0:1, :], ones_col[:, 0:1], t4s[hh][:],
                        start=True, stop=True,
                    )
                    nc.vector.reciprocal_approx_fast(
                        out=rec32[:, hh, :], in_=dps[0:1, :]
                    )
                rec_bf = sbn.tile([1, 2, 512], BF, tag="rec", name="rb", bufs=2)
                nc.vector.tensor_copy(rec_bf[:], rec32[:])
                rec_tiles[(qg, hp)] = rec_bf
                tmp = sbn.tile([128, 512], BF, tag="tmp", name="tmp", bufs=3)
                # qg3 copies are emitted after every exp, so running them on
                # the (then-idle) ACT frees the pv PSUM slot without DVE
                # queueing delay -- the next PV chain is gated on this.
                if qg == 3 or (qg == 2 and hp == 1):
                    nc.scalar.copy(tmp[:], pv[:])
                else:
                    nc.vector.tensor_copy(tmp[:], pv[:])
                tmp_tiles[(qg, hp)] = tmp

            def normalize_round(qg, hp):
                """rep-matmul + multiply -> attn_p[hp][qg] (both heads)."""
                rp = ps_misc.tile([128, 512], F32, tag="sm", name="rp")
                tmp = tmp_tiles.pop((qg, hp))
                rec_bf = rec_tiles.pop((qg, hp))
                for hh in range(2):
                    rows = slice(64 * hh, 64 * hh + 64)
                    nc.tensor.matmul(
                        rp[rows, :], ones_bf[0:1, 0:64], rec_bf[:, hh, :],
                        start=True, stop=True,
                    )
                nc.vector.tensor_mul(
                    attn_p[hp][qg][:], tmp[:], rp[:],
                )

            def outproj_chunk(qg):
                """partial out-projection rows for query group qg."""
                for tt4 in range(4):
                    tt = 4 * qg + tt4
                    o_sb = osb.tile([128, C], BF, tag="o", name="osb", bufs=3)
                    for cn in range(2):
                        op = ps_misc.tile(
                            [128, 512], F32, tag="sm", name="op"
                        )
                        for hp in range(2):
                            nc.tensor.matmul(
                                op[:],
                                attn_p[hp][qg][:, ts(tt4, 128)],
                                wout_sb[:, hp, ts(cn, 512)],
                                start=(hp == 0),
                                stop=(hp == 1),
                            )
                        # qg>=2 outproj copies are emitted after all exps
                        # (ACT queue is drained of them by then), so split
                        # the PSUM evacuation across ACT and DVE.
                        if qg >= 2 and cn == 0:
                            nc.scalar.copy(o_sb[:, ts(cn, 512)], op[:])
                        elif qg == 3:
                            nc.scalar.copy(o_sb[:, ts(cn, 512)], op[:])
                        else:
                            nc.vector.tensor_copy(o_sb[:, ts(cn, 512)], op[:])
                    # out DMAs ride sync/gpsimd only -- the scalar queue's
                    # DIRECT2D descriptor gen would head-of-line-block the
                    # exp stream on the ACT sequencer. Last chunk: halves
                    # on both queues to shorten the final-DMA tail.
                    if qg == 3:
                        # both HWDGE queues (scalar is idle post-exps):
                        # avoids the gpsimd software-DGE's ~650ns/DMA
                        # serialized descriptor generation on the tail.
                        nc.sync.dma_start(
                            out=out[ts(tt, 128), 0:512], in_=o_sb[:, 0:512]
                        )
                        nc.scalar.dma_start(
                            out=out[ts(tt, 128), 512:1024],
                            in_=o_sb[:, 512:1024],
                        )
                    else:
                        oq = nc.sync if tt % 2 == 0 else nc.gpsimd
                        oq.dma_start(out=out[ts(tt, 128), :], in_=o_sb[:])

            # ---- flash-style startup: feed ACT as early as possible ----
            # Scores-critical work is emitted (= prioritized) strictly ahead
            # of the V projections, which are pure PE filler in the ACT-paced
            # slack of rounds 1-2; pv(0,0) directly follows V.
            # round 0 (qg0, hp0): k chunk0 + q chunk0(tt0); scores chase the
            # k tt-groups as they land.
            qk_group(1, 0, 0)
            qk_group(0, 0, 0)
            # bulk loads (4MB bf16 x for V + wv + wout) split gpsimd/sync;
            # they sit FIFO behind the critical x8 loads on each queue.
            nc.gpsimd.dma_start(out=wv_sb[:], in_=wv[:])
            xv_tiles = []
            for vtt in range(4):
                xv = osb.tile([128, 8, 512], BF, tag="xv", name="xv", bufs=2)
                q = nc.gpsimd if vtt < 2 else nc.sync
                q.dma_start(out=xv[:], in_=xt[:, vtt])
                xv_tiles.append(xv)
            nc.sync.dma_start(out=wout_sb[:], in_=wout[:])
            p00 = p_alloc(0, 0)
            st_seg(0, 0, range(0, 4), p00)
            # ---- PE warmup: emitted AFTER the round-0 critical chain so
            # the scheduler treats it as gap filler (keeps the HAM clock
            # gate fed during DMA waits) instead of queue-blocking the
            # first projections.
            warm_ps = ps_misc.tile([128, 512], F32, tag="sm", name="warm")
            for i in range(17):
                nc.tensor.matmul(
                    warm_ps[0:1, 0:128], warm_src[:, 0:1], warm_src[:, 0:128],
                    start=(i == 0), stop=(i == 16),
                )
            qk_group(1, 0, 1)
            st_seg(0, 0, range(4, 8), p00)
            qk_group(1, 0, 2)
            st_seg(0, 0, range(8, 12), p00)
            qk_group(1, 0, 3)
            st_seg(0, 0, range(12, 16), p00)

            # round 1 (qg0, hp1): k chunk1 + q chunk1(tt0)
            qk_group(1, 1, 0)
            qk_group(0, 1, 0)
            p01 = p_alloc(0, 1)
            st_seg(0, 1, range(0, 4), p01)
            qk_group(1, 1, 1)
            st_seg(0, 1, range(4, 8), p01)
            qk_group(1, 1, 2)
            st_seg(0, 1, range(8, 12), p01)
            qk_group(1, 1, 3)
            st_seg(0, 1, range(12, 16), p01)

            # round 2 (qg1, hp0) scores, then V in its ACT-slack
            qk_group(0, 0, 1)
            qk_group(0, 1, 1)
            st_part(1, 0)
            for tv in range(16):
                v_group(tv)

            # ---- pipelined main stream ----
            pv_part(0, 0)
            st_part(1, 1)
            pv_part(0, 1)
            qk_group(0, 0, 2)
            qk_group(0, 1, 2)
            st_part(2, 0)
            normalize_round(0, 0)
            pv_part(1, 0)
            st_part(2, 1)
            normalize_round(0, 1)
            outproj_chunk(0)
            pv_part(1, 1)
            qk_group(0, 0, 3)
            qk_group(0, 1, 3)
            st_part(3, 0)
            normalize_round(1, 0)
            pv_part(2, 0)
            st_part(3, 1)
            normalize_round(1, 1)
            outproj_chunk(1)
            pv_part(2, 1)
            normalize_round(2, 0)
            normalize_round(2, 1)
            outproj_chunk(2)
            pv_part(3, 0)
            normalize_round(3, 0)
            pv_part(3, 1)
            normalize_round(3, 1)
            outproj_chunk(3)

    nc.compile()
    return nc


def _get_nc():
    global _NC_CACHE
    if _NC_CACHE is None:
        _NC_CACHE = _build_nc()
    return _NC_CACHE


def kernel(x, w_qkv, b_qkv, w_out, b_out):
    global LAST_RESULT
    x = np.asarray(x, dtype=np.float32)
    w_qkv = np.asarray(w_qkv, dtype=np.float32)
    b_qkv = np.asarray(b_qkv, dtype=np.float32)
    w_out = np.asarray(w_out, dtype=np.float32)
    b_out = np.asarray(b_out, dtype=np.float32)

    bf = ml_dtypes.bfloat16

    f8 = ml_dtypes.float8_e4m3fn

    def blk_w(w):  # [1024, n] -> [128, 8, n] (p, ci, n) contiguous
        n = w.shape[1]
        return np.ascontiguousarray(
            w.reshape(8, 128, n).transpose(1, 0, 2)
        ).astype(bf)

    def blk_w8(w):  # [1024, n] -> [128, 4, 2, n] DoubleRow-packed fp8
        n = w.shape[1]
        return np.ascontiguousarray(
            w.reshape(4, 2, 128, n).transpose(2, 0, 1, 3).astype(f8)
        )

    in_maps = []
    for c in range(N_CORES):
        b, g = divmod(c, 4)
        cols = slice(CH * g, CH * (g + 1))
        bq = b_qkv[0 * C + CH * g : 0 * C + CH * (g + 1)]
        bk = b_qkv[1 * C + CH * g : 1 * C + CH * (g + 1)]
        # x^T token-blocked: [p, tt, ci, 512]
        xtb = np.ascontiguousarray(
            x[b].T.astype(bf).reshape(8, 128, 4, 512).transpose(1, 2, 0, 3)
        )
        # fp8 x^T DoubleRow-packed: [p, tt, jp, ko, 512]
        x8b = np.ascontiguousarray(
            x[b].T.reshape(4, 2, 128, 4, 512).transpose(2, 3, 0, 1, 4).astype(f8)
        )
        # wout row-blocked: [p, hp, 1024]
        wob = np.ascontiguousarray(
            w_out[CH * g : CH * (g + 1), :].reshape(2, 128, C).transpose(1, 0, 2)
        ).astype(bf)
        in_maps.append(
            {
                "xt": xtb,
                "xq8": x8b,
                "wq": blk_w8(w_qkv[:, 0 * C :][:, cols]),
                "wk": blk_w8(w_qkv[:, 1 * C :][:, cols]),
                "wv": blk_w(w_qkv[:, 2 * C :][:, cols]),
                "bqt": np.ascontiguousarray(bq.reshape(2, 128).T),
                "bkt": np.ascontiguousarray(bk.reshape(2, 128).T),
                "wout": wob,
            }
        )

    nc = _get_nc()
    LAST_RESULT = bass_utils.run_bass_kernel_spmd(
        nc, in_maps, core_ids=list(range(N_CORES))
    )

    full = np.zeros((B, T, C), dtype=np.float32)
    # bias folded once on the host: b_out plus the V-bias pushed through
    # w_out (normalized attention rows sum to 1, so bv contributes exactly
    # bv @ w_out to every token)
    full += b_out + b_qkv[2 * C : 3 * C] @ w_out
    for c in range(N_CORES):
        b = c // 4
        full[b] += LAST_RESULT.results[c]["out"].astype(np.float32)
    return full

# nonce 1

# nonce 2

# nonce 3



# revision 43
# speedup vs baseline: 1.0211x; 1.0211x over previous
"""Multi-head self-attention on 8 Trainium2 NeuronCores (Bass/Tile).

Problem: x[2,2048,1024] -> MHA(16 heads, d_head 64) -> out[2,2048,1024].

Sharding (batch x head-group, Megatron-ish, collective-free):
  core c (0..7): batch b = c//4, head group g = c%4 (heads 4g..4g+3).
  Each core computes q/k/v projections for its 4 heads over its batch,
  attention for those heads, and a PARTIAL output projection
  attn_local[256ch] @ w_out[256ch rows] over the full sequence. The host
  sums the 4 partials per batch (the Megatron row-parallel all-reduce is
  folded into the unshard step; b_out and the V-bias term bv @ w_out are
  added once on the host -- exact, since softmax rows sum to 1).

On-core layout (TensorE compute in bf16, fp32 PSUM accumulation):
  - ACT (exp for softmax) is the bottleneck engine: 16.8M exps/core ~=
    147us of ACT instruction time. The schedule saturates ACT from the
    earliest possible point after the ~7us engine-boot preamble:
    * all inputs arrive in host-pre-blocked layouts that are contiguous
      per SBUF partition (cheap HWDGE triggers, 4-8KB DMA lines), split
      across the sync and scalar queues with x^T token-sliced;
    * k/q chunk-0 projections are emitted first so scores round 0 feeds
      ACT immediately; V projections and the remaining q/k chunks are
      pure PE filler behind ACT pacing.
  - exp p-tiles are paired per round ([128, 2heads, 8kp, 1024]) with 3
    rotating buffers = 3 rounds in flight, so round r+2's exps never
    wait on round r's PV consumption.
  - qT/kT in [channel, t] layout: scores^T = kT.T @ qT with the two
    heads of a chunk in partitions 0-63/64-127 -> concurrent K=64
    matmuls in disjoint PE row groups.
  - softmax: scores^T [128ki, qi] -> ACT exp (PSUM->SBUF bf16,
    scale=1/8 folded, no max subtraction: |s|/8 <= ~2).
  - PV: attn^T = V.T @ P~ as column-tiled concurrent M=64 head pairs;
    denominators via DVE bf16 add-tree + K=128 ones-matmul fold,
    reciprocal_approx_fast straight off PSUM; normalize one round
    behind PV (rep-matmul broadcast + DVE mul); out-projection +
    output DMA per query group.
"""

import numpy as np
import ml_dtypes

import concourse.bass as bass
import concourse.mybir as mybir
import concourse.tile as tile
from concourse import bacc
from concourse import bass_utils
from concourse.bass import ts

BF = mybir.dt.bfloat16
F32 = mybir.dt.float32

B, T, C = 2, 2048, 1024
H, DH = 16, 64
N_CORES = 8
HG = 4  # heads per core
CH = HG * DH  # 256 channels per core

LAST_RESULT = None  # BassKernelResults of the most recent run (for profiling)
_NC_CACHE = None


def _build_nc():
    nc = bacc.Bacc(
        "TRN2", target_bir_lowering=False, debug=False, num_devices=N_CORES
    )

    # host-pre-blocked layouts: every tensor is contiguous along its SBUF
    # partition's free dim, so each DMA is 128 x (one fat line).
    F8 = mybir.dt.float8e4
    xt = nc.dram_tensor("xt", [128, 4, 8, 512], BF, kind="ExternalInput")
    xq8 = nc.dram_tensor("xq8", [128, 4, 4, 2, 512], F8, kind="ExternalInput")
    wq = nc.dram_tensor("wq", [128, 4, 2, CH], F8, kind="ExternalInput")
    wk = nc.dram_tensor("wk", [128, 4, 2, CH], F8, kind="ExternalInput")
    wv = nc.dram_tensor("wv", [128, 8, CH], BF, kind="ExternalInput")
    bqt = nc.dram_tensor("bqt", [128, 2], F32, kind="ExternalInput")
    bkt = nc.dram_tensor("bkt", [128, 2], F32, kind="ExternalInput")
    wout = nc.dram_tensor("wout", [128, 2, C], BF, kind="ExternalInput")
    out = nc.dram_tensor("out", [T, C], BF, kind="ExternalOutput")

    with tile.TileContext(nc) as tc:
        with (
            tc.tile_pool(name="persist", bufs=1) as persist,
            tc.tile_pool(name="consts", bufs=1) as consts,
            tc.tile_pool(name="sbn", bufs=2) as sbn,
            tc.tile_pool(name="osb", bufs=3) as osb,
            tc.tile_pool(name="ps_st", bufs=2, space="PSUM") as ps_st,
            tc.tile_pool(name="ps_pv", bufs=1, space="PSUM") as ps_pv,
            tc.tile_pool(name="ps_misc", bufs=3, space="PSUM") as ps_misc,
        ):
            ones_bf = consts.tile([1, 128], BF)
            nc.vector.memset(ones_bf[:], 1.0)
            ones_col = consts.tile([128, 1], BF)
            nc.vector.memset(ones_col[:], 1.0)


            # fp8 x (DoubleRow-packed) for q/k; bf16 x arrives per-tt into
            # a rotating buffer for the V projections.
            x8_sb = persist.tile([128, 4, 4, 2, 512], F8, tag="x8")
            wq_sb = persist.tile([128, 4, 2, CH], F8, tag="wq")
            wk_sb = persist.tile([128, 4, 2, CH], F8, tag="wk")
            wv_sb = persist.tile([128, 8, CH], BF, tag="wv")
            wout_sb = persist.tile([128, 2, C], BF, tag="wout")
            bqt_sb = consts.tile([128, 2], F32)
            bkt_sb = consts.tile([128, 2], F32)

            # ---- input DMA: 3-way queue split, critical-first ----
            # per-queue bandwidth is only ~65-160GB/s, so the x8 token
            # blocks are spread over all three queues (tt0+tt1 sync,
            # tt2 scalar, tt3 gpsimd): all of x8 lands by ~6us instead
            # of ~14us, so the kT chunk projections never starve the
            # score/exp stream. Bulk V-path loads queue FIFO behind the
            # critical loads on their queue (self-gating).
            nc.sync.dma_start(out=x8_sb[:, 0], in_=xq8[:, 0])
            nc.scalar.dma_start(out=bkt_sb[:], in_=bkt[:])
            nc.scalar.dma_start(out=wk_sb[:], in_=wk[:])
            nc.gpsimd.dma_start(out=bqt_sb[:], in_=bqt[:])
            nc.gpsimd.dma_start(out=wq_sb[:], in_=wq[:])
            nc.sync.dma_start(out=x8_sb[:, 1], in_=xq8[:, 1])
            nc.scalar.dma_start(out=x8_sb[:, 2], in_=xq8[:, 2])
            nc.gpsimd.dma_start(out=x8_sb[:, 3], in_=xq8[:, 3])

            warm_src = consts.tile([128, 512], BF)
            nc.vector.memset(warm_src[:], 0.0)

            # ---- persistent activations ----
            # qkT[:, 0:2, :] = qT chunks (hp), [:, 2:4, :] = kT chunks;
            # chunk hp rows 0-63 = head 2hp, rows 64-127 = head 2hp+1.
            qkT = persist.tile([128, 4, T], BF, tag="qkT")
            vext = persist.tile([128, T // 128, HG, DH], BF, tag="vext")
            attn_p = [
                [
                    persist.tile(
                        [128, 512], BF, tag=f"attnp{hp}_{qg}",
                        name=f"attnp{hp}_{qg}",
                    )
                    for qg in range(4)
                ]
                for hp in range(2)
            ]

            def qk_group(w_i, co, tt):
                """one [128,512] tile of qT (w_i=0) or kT (w_i=1), chunk co"""
                wsb = wq_sb if w_i == 0 else wk_sb
                bias_sb = bqt_sb if w_i == 0 else bkt_sb
                qp = ps_misc.tile([128, 512], F32, tag="sm", name="qp")
                for jp in range(4):
                    nc.tensor.matmul(
                        qp[:],
                        wsb[:, jp, :, ts(co, 128)],
                        x8_sb[:, tt, jp, :, :],
                        perf_mode=mybir.MatmulPerfMode.DoubleRow,
                        start=(jp == 0),
                        stop=(jp == 3),
                    )
                # bias-add + cast on the DVE (keeps the ACT queue for exps)
                nc.vector.tensor_scalar_add(
                    qkT[:, 2 * w_i + co, ts(tt, 512)],
                    qp[:],
                    bias_sb[:, co : co + 1],
                )

            def v_group(tv):
                vp = ps_misc.tile([128, CH], F32, tag="sm", name="vp")
                for ci in range(8):
                    nc.tensor.matmul(
                        vp[:],
                        xv_tiles[tv // 4][:, ci, ts(tv % 4, 128)],
                        wv_sb[:, ci, :],
                        start=(ci == 0),
                        stop=(ci == 7),
                    )
                nc.vector.tensor_copy(
                    vext[:, tv, :, :],
                    vp[:].rearrange("p (h d) -> p h d", h=HG),
                )

            p_tiles = {}
            rec_tiles = {}
            tmp_tiles = {}

            def p_alloc(qg, hp):
                # paired tile: [p, head(A/B), kp, 1024]; one pool slot per
                # round -> bufs=3 keeps 3 rounds of exps live.
                p = osb.tile([128, 2, 8, 1024], BF, tag="p", bufs=3, name="p")
                p_tiles[(qg, hp)] = p
                return p

            def st_seg(qg, hp, kis, p):
                """scores^T + exp for head pair hp, query group qg, ki range.

                One PSUM tile holds BOTH heads' scores for a 128-key block
                (A in cols 0:512, B in 512:1024): the two K=64 matmuls hit
                disjoint PE row groups (0/64) and are adjacent in issue
                order, so they execute concurrently (row tiling)."""
                qs = ts(qg, 512)
                for ki in kis:
                    stp = ps_st.tile([128, 1024], F32, tag="st", name="st")
                    nc.tensor.matmul(
                        stp[:, 0:512],
                        qkT[0:64, 2 + hp, ts(ki, 128)],
                        qkT[0:64, hp, qs],
                        start=True, stop=True,
                    )
                    nc.tensor.matmul(
                        stp[:, 512:1024],
                        qkT[64:128, 2 + hp, ts(ki, 128)],
                        qkT[64:128, hp, qs],
                        start=True, stop=True,
                    )
                    # exp out: head A -> p[:, 0, ki//2, (ki%2)*512 :],
                    # head B -> p[:, 1, ...] (strided 3D AP, one ACTIVATE)
                    nc.scalar.activation(
                        p[:, 0:2, ki // 2, ts(ki % 2, 512)], stp[:],
                        mybir.ActivationFunctionType.Exp, scale=1.0 / 8.0,
                    )

            def st_part(qg, hp):
                p = p_alloc(qg, hp)
                st_seg(qg, hp, range(16), p)

            def pv_part(qg, hp):
                p = p_tiles.pop((qg, hp))
                t4s = {}
                for hh in range(2):
                    # incremental 4-way tree: s_i = kp(2i)+kp(2i+1) can run
                    # as soon as those exps land, so the post-last-exp
                    # critical chain is only s3 -> u1 -> v -> t4 (~2us)
                    # instead of a full 4-level tree (~4.4us).
                    s = []
                    for i in range(4):
                        si = sbn.tile(
                            [128, 1024], BF, tag=f"s{i}", name=f"s{i}", bufs=1
                        )
                        nc.vector.tensor_add(
                            si[:], p[:, hh, 2 * i, :], p[:, hh, 2 * i + 1, :]
                        )
                        s.append(si)
                    u0 = sbn.tile([128, 1024], BF, tag="u0", name="u0", bufs=1)
                    nc.vector.tensor_add(u0[:], s[0][:], s[1][:])
                    u1 = sbn.tile([128, 1024], BF, tag="u1", name="u1", bufs=1)
                    nc.vector.tensor_add(u1[:], s[2][:], s[3][:])
                    v3 = sbn.tile([128, 1024], BF, tag="v3", name="v3", bufs=1)
                    nc.vector.tensor_add(v3[:], u0[:], u1[:])
                    t4 = sbn.tile([128, 512], BF, tag="t4", name="t4", bufs=2)
                    nc.vector.tensor_add(
                        t4[:], v3[:, 0:512], v3[:, 512:1024]
                    )
                    t4s[hh] = t4
                pv = ps_pv.tile([128, 512], F32, tag="pv", name="pv")
                for ki in range(16):
                    for hh in range(2):
                        h = 2 * hp + hh
                        nc.tensor.matmul(
                            pv[64 * hh : 64 * hh + 64, :],
                            vext[:, ki, h, :],
                            p[:, hh, ki // 2, ts(ki % 2, 512)],
                            start=(ki == 0),
                            stop=(ki == 15),
                        )
                # both heads' reciprocals free-packed in one [1,2,512]
                # tile so the bf16 cast runs once per part.
                rec32 = sbn.tile([1, 2, 512], F32, tag="rec32", name="rc", bufs=1)
                for hh in range(2):
                    dps = ps_misc.tile([128, 512], F32, tag="sm", name="dps")
                    nc.tensor.matmul(
                        dps[0:1, :], ones_col[:, 0:1], t4s[hh][:],
                        start=True, stop=True,
                    )
                    nc.vector.reciprocal_approx_fast(
                        out=rec32[:, hh, :], in_=dps[0:1, :]
                    )
                rec_bf = sbn.tile([1, 2, 512], BF, tag="rec", name="rb", bufs=2)
                nc.vector.tensor_copy(rec_bf[:], rec32[:])
                rec_tiles[(qg, hp)] = rec_bf
                tmp = sbn.tile([128, 512], BF, tag="tmp", name="tmp", bufs=3)
                # qg3 copies are emitted after every exp, so running them on
                # the (then-idle) ACT frees the pv PSUM slot without DVE
                # queueing delay -- the next PV chain is gated on this.
                if qg == 3:
                    nc.scalar.copy(tmp[:], pv[:])
                else:
                    nc.vector.tensor_copy(tmp[:], pv[:])
                tmp_tiles[(qg, hp)] = tmp

            def normalize_round(qg, hp):
                """rep-matmul + multiply -> attn_p[hp][qg] (both heads)."""
                rp = ps_misc.tile([128, 512], F32, tag="sm", name="rp")
                tmp = tmp_tiles.pop((qg, hp))
                rec_bf = rec_tiles.pop((qg, hp))
                for hh in range(2):
                    rows = slice(64 * hh, 64 * hh + 64)
                    nc.tensor.matmul(
                        rp[rows, :], ones_bf[0:1, 0:64], rec_bf[:, hh, :],
                        start=True, stop=True,
                    )
                nc.vector.tensor_mul(
                    attn_p[hp][qg][:], tmp[:], rp[:],
                )

            def outproj_chunk(qg):
                """partial out-projection rows for query group qg."""
                for tt4 in range(4):
                    tt = 4 * qg + tt4
                    o_sb = osb.tile([128, C], BF, tag="o", name="osb", bufs=3)
                    for cn in range(2):
                        op = ps_misc.tile(
                            [128, 512], F32, tag="sm", name="op"
                        )
                        for hp in range(2):
                            nc.tensor.matmul(
                                op[:],
                                attn_p[hp][qg][:, ts(tt4, 128)],
                                wout_sb[:, hp, ts(cn, 512)],
                                start=(hp == 0),
                                stop=(hp == 1),
                            )
                        # qg>=2 outproj copies are emitted after all exps
                        # (ACT queue is drained of them by then), so split
                        # the PSUM evacuation across ACT and DVE.
                        if qg >= 2 and cn == 0:
                            nc.scalar.copy(o_sb[:, ts(cn, 512)], op[:])
                        else:
                            nc.vector.tensor_copy(o_sb[:, ts(cn, 512)], op[:])
                    # out DMAs ride sync/gpsimd only -- the scalar queue's
                    # DIRECT2D descriptor gen would head-of-line-block the
                    # exp stream on the ACT sequencer. Last chunk: halves
                    # on both queues to shorten the final-DMA tail.
                    if qg == 3:
                        # both HWDGE queues (scalar is idle post-exps):
                        # avoids the gpsimd software-DGE's ~650ns/DMA
                        # serialized descriptor generation on the tail.
                        nc.sync.dma_start(
                            out=out[ts(tt, 128), 0:512], in_=o_sb[:, 0:512]
                        )
                        nc.scalar.dma_start(
                            out=out[ts(tt, 128), 512:1024],
                            in_=o_sb[:, 512:1024],
                        )
                    else:
                        oq = nc.sync if tt % 2 == 0 else nc.gpsimd
                        oq.dma_start(out=out[ts(tt, 128), :], in_=o_sb[:])

            # ---- flash-style startup: feed ACT as early as possible ----
            # Scores-critical work is emitted (= prioritized) strictly ahead
            # of the V projections, which are pure PE filler in the ACT-paced
            # slack of rounds 1-2; pv(0,0) directly follows V.
            # round 0 (qg0, hp0): k chunk0 + q chunk0(tt0); scores chase the
            # k tt-groups as they land.
            qk_group(1, 0, 0)
            qk_group(0, 0, 0)
            # bulk loads (4MB bf16 x for V + wv + wout) split gpsimd/sync;
            # they sit FIFO behind the critical x8 loads on each queue.
            nc.gpsimd.dma_start(out=wv_sb[:], in_=wv[:])
            xv_tiles = []
            for vtt in range(4):
                xv = osb.tile([128, 8, 512], BF, tag="xv", name="xv", bufs=2)
                q = nc.gpsimd if vtt < 2 else nc.sync
                q.dma_start(out=xv[:], in_=xt[:, vtt])
                xv_tiles.append(xv)
            nc.sync.dma_start(out=wout_sb[:], in_=wout[:])
            p00 = p_alloc(0, 0)
            st_seg(0, 0, range(0, 4), p00)
            # ---- PE warmup: emitted AFTER the round-0 critical chain so
            # the scheduler treats it as gap filler (keeps the HAM clock
            # gate fed during DMA waits) instead of queue-blocking the
            # first projections.
            warm_ps = ps_misc.tile([128, 512], F32, tag="sm", name="warm")
            for i in range(17):
                nc.tensor.matmul(
                    warm_ps[0:1, 0:128], warm_src[:, 0:1], warm_src[:, 0:128],
                    start=(i == 0), stop=(i == 16),
                )
            qk_group(1, 0, 1)
            st_seg(0, 0, range(4, 8), p00)
            qk_group(1, 0, 2)
            st_seg(0, 0, range(8, 12), p00)
            qk_group(1, 0, 3)
            st_seg(0, 0, range(12, 16), p00)

            # round 1 (qg0, hp1): k chunk1 + q chunk1(tt0)
            qk_group(1, 1, 0)
            qk_group(0, 1, 0)
            p01 = p_alloc(0, 1)
            st_seg(0, 1, range(0, 4), p01)
            qk_group(1, 1, 1)
            st_seg(0, 1, range(4, 8), p01)
            qk_group(1, 1, 2)
            st_seg(0, 1, range(8, 12), p01)
            qk_group(1, 1, 3)
            st_seg(0, 1, range(12, 16), p01)

            # round 2 (qg1, hp0) scores, then V in its ACT-slack
            qk_group(0, 0, 1)
            qk_group(0, 1, 1)
            st_part(1, 0)
            for tv in range(16):
                v_group(tv)

            # ---- pipelined main stream ----
            pv_part(0, 0)
            st_part(1, 1)
            pv_part(0, 1)
            qk_group(0, 0, 2)
            qk_group(0, 1, 2)
            st_part(2, 0)
            normalize_round(0, 0)
            pv_part(1, 0)
            st_part(2, 1)
            normalize_round(0, 1)
            outproj_chunk(0)
            pv_part(1, 1)
            qk_group(0, 0, 3)
            qk_group(0, 1, 3)
            st_part(3, 0)
            normalize_round(1, 0)
            pv_part(2, 0)
            st_part(3, 1)
            normalize_round(1, 1)
            outproj_chunk(1)
            pv_part(2, 1)
            normalize_round(2, 0)
            normalize_round(2, 1)
            outproj_chunk(2)
            pv_part(3, 0)
            normalize_round(3, 0)
            pv_part(3, 1)
            normalize_round(3, 1)
            outproj_chunk(3)

    nc.compile()
    return nc


def _get_nc():
    global _NC_CACHE
    if _NC_CACHE is None:
        _NC_CACHE = _build_nc()
    return _NC_CACHE


def kernel(x, w_qkv, b_qkv, w_out, b_out):
    global LAST_RESULT
    x = np.asarray(x, dtype=np.float32)
    w_qkv = np.asarray(w_qkv, dtype=np.float32)
    b_qkv = np.asarray(b_qkv, dtype=np.float32)
    w_out = np.asarray(w_out, dtype=np.float32)
    b_out = np.asarray(b_out, dtype=np.float32)

    bf = ml_dtypes.bfloat16

    f8 = ml_dtypes.float8_e4m3fn

    def blk_w(w):  # [1024, n] -> [128, 8, n] (p, ci, n) contiguous
        n = w.shape[1]
        return np.ascontiguousarray(
            w.reshape(8, 128, n).transpose(1, 0, 2)
        ).astype(bf)

    def blk_w8(w):  # [1024, n] -> [128, 4, 2, n] DoubleRow-packed fp8
        n = w.shape[1]
        return np.ascontiguousarray(
            w.reshape(4, 2, 128, n).transpose(2, 0, 1, 3).astype(f8)
        )

    in_maps = []
    for c in range(N_CORES):
        b, g = divmod(c, 4)
        cols = slice(CH * g, CH * (g + 1))
        bq = b_qkv[0 * C + CH * g : 0 * C + CH * (g + 1)]
        bk = b_qkv[1 * C + CH * g : 1 * C + CH * (g + 1)]
        # x^T token-blocked: [p, tt, ci, 512]
        xtb = np.ascontiguousarray(
            x[b].T.astype(bf).reshape(8, 128, 4, 512).transpose(1, 2, 0, 3)
        )
        # fp8 x^T DoubleRow-packed: [p, tt, jp, ko, 512]
        x8b = np.ascontiguousarray(
            x[b].T.reshape(4, 2, 128, 4, 512).transpose(2, 3, 0, 1, 4).astype(f8)
        )
        # wout row-blocked: [p, hp, 1024]
        wob = np.ascontiguousarray(
            w_out[CH * g : CH * (g + 1), :].reshape(2, 128, C).transpose(1, 0, 2)
        ).astype(bf)
        in_maps.append(
            {
                "xt": xtb,
                "xq8": x8b,
                "wq": blk_w8(w_qkv[:, 0 * C :][:, cols]),
                "wk": blk_w8(w_qkv[:, 1 * C :][:, cols]),
                "wv": blk_w(w_qkv[:, 2 * C :][:, cols]),
                "bqt": np.ascontiguousarray(bq.reshape(2, 128).T),
                "bkt": np.ascontiguousarray(bk.reshape(2, 128).T),
                "wout": wob,
            }
        )

    nc = _get_nc()
    LAST_RESULT = bass_utils.run_bass_kernel_spmd(
        nc, in_maps, core_ids=list(range(N_CORES))
    )

    full = np.zeros((B, T, C), dtype=np.float32)
    # bias folded once on the host: b_out plus the V-bias pushed through
    # w_out (normalized attention rows sum to 1, so bv contributes exactly
    # bv @ w_out to every token)
    full += b_out + b_qkv[2 * C : 3 * C] @ w_out
    for c in range(N_CORES):
        b = c // 4
        full[b] += LAST_RESULT.results[c]["out"].astype(np.float32)
    return full

# nonce 1

# nonce 2

# nonce 3



# revision 45
# speedup vs baseline: 1.0467x; 1.0250x over previous
"""Multi-head self-attention on 8 Trainium2 NeuronCores (Bass/Tile).

Problem: x[2,2048,1024] -> MHA(16 heads, d_head 64) -> out[2,2048,1024].

Sharding (batch x head-group, Megatron-ish, collective-free):
  core c (0..7): batch b = c//4, head group g = c%4 (heads 4g..4g+3).
  Each core computes q/k/v projections for its 4 heads over its batch,
  attention for those heads, and a PARTIAL output projection
  attn_local[256ch] @ w_out[256ch rows] over the full sequence. The host
  sums the 4 partials per batch (the Megatron row-parallel all-reduce is
  folded into the unshard step; b_out and the V-bias term bv @ w_out are
  added once on the host -- exact, since softmax rows sum to 1).

On-core layout (TensorE compute in bf16, fp32 PSUM accumulation).
The kernel is three-way balanced: ACT exps ~132us (128 x [128,1024]
EXP at ~1028ns), PE ~154us busy, DVE ~133us; the exp stream runs
stall-free once started:
    * all inputs arrive in host-pre-blocked layouts that are contiguous
      per SBUF partition (cheap HWDGE triggers, 4-8KB DMA lines), split
      across the sync and scalar queues with x^T token-sliced;
    * k/q chunk-0 projections are emitted first so scores round 0 feeds
      ACT immediately; V projections and the remaining q/k chunks are
      pure PE filler behind ACT pacing.
  - exp p-tiles are paired per round ([128, 2heads, 8kp, 1024]) with 3
    rotating buffers = 3 rounds in flight, so round r+2's exps never
    wait on round r's PV consumption.
  - qT/kT in [channel, t] layout. Both heads' scores for one 128-key
    block share ONE PSUM tile (A cols 0:512, B cols 512:1024): the two
    K=64 matmuls hit disjoint PE row tiles (0,0)/(64,0), are adjacent
    in issue order, and execute CONCURRENTLY (row tiling, ~2x on the
    scores phase; verified in trace: pair ~320ns vs 533ns serial).
  - softmax: scores^T [128ki, A|B] -> one ACT exp per key block
    (PSUM->SBUF bf16 via strided [128,2,512] out AP, scale=1/8 folded,
    no max subtraction: |s|/8 <= ~2).
  - PV: attn^T = V.T @ P~ as column-tiled concurrent M=64 head pairs;
    denominators via an INCREMENTAL DVE bf16 add-tree (s_i = kp pair
    sums run as exps land; post-last-exp chain is only s3->u1->v->t4)
    + K=128 ones-matmul fold; both heads' reciprocals free-packed in
    one [1,2,512] tile (single bf16 cast); normalize one round behind
    PV (rep-matmul broadcast + DVE mul); out-projection + output DMA
    per query group (sync/gpsimd queues only -- scalar-queue DMA
    descriptor gen would head-of-line-block the exp stream; the last
    chunk's DMAs are split in column halves across both queues).
  - tail: qg3 tmp/o_sb PSUM evacuations ride the then-idle ACT queue
    (they are emitted after every exp, so they cannot block them).
  - PE warmup is emitted AFTER round-0's critical chain so the Tile
    scheduler treats it as gap filler for the HAM clock gate instead
    of queue-blocking the first projections.
"""

import numpy as np
import ml_dtypes

import concourse.bass as bass
import concourse.mybir as mybir
import concourse.tile as tile
from concourse import bacc
from concourse import bass_utils
from concourse.bass import ts

BF = mybir.dt.bfloat16
F32 = mybir.dt.float32

B, T, C = 2, 2048, 1024
H, DH = 16, 64
N_CORES = 8
HG = 4  # heads per core
CH = HG * DH  # 256 channels per core

LAST_RESULT = None  # BassKernelResults of the most recent run (for profiling)
_NC_CACHE = None


def _build_nc():
    nc = bacc.Bacc(
        "TRN2", target_bir_lowering=False, debug=False, num_devices=N_CORES
    )

    # host-pre-blocked layouts: every tensor is contiguous along its SBUF
    # partition's free dim, so each DMA is 128 x (one fat line).
    F8 = mybir.dt.float8e4
    xt = nc.dram_tensor("xt", [128, 4, 8, 512], BF, kind="ExternalInput")
    xq8 = nc.dram_tensor("xq8", [128, 4, 4, 2, 512], F8, kind="ExternalInput")
    wq = nc.dram_tensor("wq", [128, 4, 2, CH], F8, kind="ExternalInput")
    wk = nc.dram_tensor("wk", [128, 4, 2, CH], F8, kind="ExternalInput")
    wv = nc.dram_tensor("wv", [128, 8, CH], BF, kind="ExternalInput")
    bqt = nc.dram_tensor("bqt", [128, 2], F32, kind="ExternalInput")
    bkt = nc.dram_tensor("bkt", [128, 2], F32, kind="ExternalInput")
    wout = nc.dram_tensor("wout", [128, 2, C], BF, kind="ExternalInput")
    out = nc.dram_tensor("out", [T, C], BF, kind="ExternalOutput")

    with tile.TileContext(nc) as tc:
        with (
            tc.tile_pool(name="persist", bufs=1) as persist,
            tc.tile_pool(name="consts", bufs=1) as consts,
            tc.tile_pool(name="sbn", bufs=2) as sbn,
            tc.tile_pool(name="osb", bufs=3) as osb,
            tc.tile_pool(name="ps_st", bufs=2, space="PSUM") as ps_st,
            tc.tile_pool(name="ps_pv", bufs=1, space="PSUM") as ps_pv,
            tc.tile_pool(name="ps_misc", bufs=3, space="PSUM") as ps_misc,
        ):
            ones_bf = consts.tile([1, 128], BF)
            nc.vector.memset(ones_bf[:], 1.0)
            ones_col = consts.tile([128, 1], BF)
            nc.vector.memset(ones_col[:], 1.0)


            # fp8 x (DoubleRow-packed) for q/k; bf16 x arrives per-tt into
            # a rotating buffer for the V projections.
            x8_sb = persist.tile([128, 4, 4, 2, 512], F8, tag="x8")
            wq_sb = persist.tile([128, 4, 2, CH], F8, tag="wq")
            wk_sb = persist.tile([128, 4, 2, CH], F8, tag="wk")
            wv_sb = persist.tile([128, 8, CH], BF, tag="wv")
            wout_sb = persist.tile([128, 2, C], BF, tag="wout")
            bqt_sb = consts.tile([128, 2], F32)
            bkt_sb = consts.tile([128, 2], F32)

            # ---- input DMA: 3-way queue split, critical-first ----
            # per-queue bandwidth is only ~65-160GB/s, so the x8 token
            # blocks are spread over all three queues (tt0+tt1 sync,
            # tt2 scalar, tt3 gpsimd): all of x8 lands by ~6us instead
            # of ~14us, so the kT chunk projections never starve the
            # score/exp stream. Bulk V-path loads queue FIFO behind the
            # critical loads on their queue (self-gating).
            nc.sync.dma_start(out=x8_sb[:, 0], in_=xq8[:, 0])
            nc.scalar.dma_start(out=bkt_sb[:], in_=bkt[:])
            nc.scalar.dma_start(out=wk_sb[:], in_=wk[:])
            nc.gpsimd.dma_start(out=bqt_sb[:], in_=bqt[:])
            nc.gpsimd.dma_start(out=wq_sb[:], in_=wq[:])
            nc.sync.dma_start(out=x8_sb[:, 1], in_=xq8[:, 1])
            nc.scalar.dma_start(out=x8_sb[:, 2], in_=xq8[:, 2])
            nc.gpsimd.dma_start(out=x8_sb[:, 3], in_=xq8[:, 3])

            warm_src = consts.tile([128, 512], BF)
            nc.vector.memset(warm_src[:], 0.0)

            # ---- persistent activations ----
            # qkT[:, 0:2, :] = qT chunks (hp), [:, 2:4, :] = kT chunks;
            # chunk hp rows 0-63 = head 2hp, rows 64-127 = head 2hp+1.
            qkT = persist.tile([128, 4, T], BF, tag="qkT")
            vext = persist.tile([128, T // 128, HG, DH], BF, tag="vext")
            attn_p = [
                [
                    persist.tile(
                        [128, 512], BF, tag=f"attnp{hp}_{qg}",
                        name=f"attnp{hp}_{qg}",
                    )
                    for qg in range(4)
                ]
                for hp in range(2)
            ]

            def qk_group(w_i, co, tt):
                """one [128,512] tile of qT (w_i=0) or kT (w_i=1), chunk co"""
                wsb = wq_sb if w_i == 0 else wk_sb
                bias_sb = bqt_sb if w_i == 0 else bkt_sb
                qp = ps_misc.tile([128, 512], F32, tag="sm", name="qp")
                for jp in range(4):
                    nc.tensor.matmul(
                        qp[:],
                        wsb[:, jp, :, ts(co, 128)],
                        x8_sb[:, tt, jp, :, :],
                        perf_mode=mybir.MatmulPerfMode.DoubleRow,
                        start=(jp == 0),
                        stop=(jp == 3),
                    )
                # bias-add + cast on the DVE (keeps the ACT queue for exps)
                nc.vector.tensor_scalar_add(
                    qkT[:, 2 * w_i + co, ts(tt, 512)],
                    qp[:],
                    bias_sb[:, co : co + 1],
                )

            def v_group(tv):
                vp = ps_misc.tile([128, CH], F32, tag="sm", name="vp")
                for ci in range(8):
                    nc.tensor.matmul(
                        vp[:],
                        xv_tiles[tv // 4][:, ci, ts(tv % 4, 128)],
                        wv_sb[:, ci, :],
                        start=(ci == 0),
                        stop=(ci == 7),
                    )
                nc.vector.tensor_copy(
                    vext[:, tv, :, :],
                    vp[:].rearrange("p (h d) -> p h d", h=HG),
                )

            p_tiles = {}
            rec_tiles = {}
            tmp_tiles = {}

            def p_alloc(qg, hp):
                # paired tile: [p, head(A/B), kp, 1024]; one pool slot per
                # round -> bufs=3 keeps 3 rounds of exps live.
                p = osb.tile([128, 2, 8, 1024], BF, tag="p", bufs=3, name="p")
                p_tiles[(qg, hp)] = p
                return p

            def st_seg(qg, hp, kis, p):
                """scores^T + exp for head pair hp, query group qg, ki range.

                One PSUM tile holds BOTH heads' scores for a 128-key block
                (A in cols 0:512, B in 512:1024): the two K=64 matmuls hit
                disjoint PE row groups (0/64) and are adjacent in issue
                order, so they execute concurrently (row tiling)."""
                qs = ts(qg, 512)
                for ki in kis:
                    stp = ps_st.tile([128, 1024], F32, tag="st", name="st")
                    nc.tensor.matmul(
                        stp[:, 0:512],
                        qkT[0:64, 2 + hp, ts(ki, 128)],
                        qkT[0:64, hp, qs],
                        start=True, stop=True,
                    )
                    nc.tensor.matmul(
                        stp[:, 512:1024],
                        qkT[64:128, 2 + hp, ts(ki, 128)],
                        qkT[64:128, hp, qs],
                        start=True, stop=True,
                    )
                    # exp out: head A -> p[:, 0, ki//2, (ki%2)*512 :],
                    # head B -> p[:, 1, ...] (strided 3D AP, one ACTIVATE)
                    nc.scalar.activation(
                        p[:, 0:2, ki // 2, ts(ki % 2, 512)], stp[:],
                        mybir.ActivationFunctionType.Exp, scale=1.0 / 8.0,
                    )

            def st_part(qg, hp):
                p = p_alloc(qg, hp)
                st_seg(qg, hp, range(16), p)

            def pv_part(qg, hp):
                p = p_tiles.pop((qg, hp))
                t4s = {}
                for hh in range(2):
                    # incremental 4-way tree: s_i = kp(2i)+kp(2i+1) can run
                    # as soon as those exps land, so the post-last-exp
                    # critical chain is only s3 -> u1 -> v -> t4 (~2us)
                    # instead of a full 4-level tree (~4.4us).
                    s = []
                    for i in range(4):
                        si = sbn.tile(
                            [128, 1024], BF, tag=f"s{i}", name=f"s{i}", bufs=1
                        )
                        nc.vector.tensor_add(
                            si[:], p[:, hh, 2 * i, :], p[:, hh, 2 * i + 1, :]
                        )
                        s.append(si)
                    u0 = sbn.tile([128, 1024], BF, tag="u0", name="u0", bufs=1)
                    nc.vector.tensor_add(u0[:], s[0][:], s[1][:])
                    u1 = sbn.tile([128, 1024], BF, tag="u1", name="u1", bufs=1)
                    nc.vector.tensor_add(u1[:], s[2][:], s[3][:])
                    v3 = sbn.tile([128, 1024], BF, tag="v3", name="v3", bufs=1)
                    nc.vector.tensor_add(v3[:], u0[:], u1[:])
                    t4 = sbn.tile([128, 512], BF, tag="t4", name="t4", bufs=2)
                    nc.vector.tensor_add(
                        t4[:], v3[:, 0:512], v3[:, 512:1024]
                    )
                    t4s[hh] = t4
                pv = ps_pv.tile([128, 512], F32, tag="pv", name="pv")
                for ki in range(16):
                    for hh in range(2):
                        h = 2 * hp + hh
                        nc.tensor.matmul(
                            pv[64 * hh : 64 * hh + 64, :],
                            vext[:, ki, h, :],
                            p[:, hh, ki // 2, ts(ki % 2, 512)],
                            start=(ki == 0),
                            stop=(ki == 15),
                        )
                # both heads' reciprocals free-packed in one [1,2,512]
                # tile so the bf16 cast runs once per part.
                rec32 = sbn.tile([1, 2, 512], F32, tag="rec32", name="rc", bufs=1)
                for hh in range(2):
                    dps = ps_misc.tile([128, 512], F32, tag="sm", name="dps")
                    nc.tensor.matmul(
                        dps[0:1, :], ones_col[:, 0:1], t4s[hh][:],
                        start=True, stop=True,
                    )
                    nc.vector.reciprocal_approx_fast(
                        out=rec32[:, hh, :], in_=dps[0:1, :]
                    )
                rec_bf = sbn.tile([1, 2, 512], BF, tag="rec", name="rb", bufs=2)
                nc.vector.tensor_copy(rec_bf[:], rec32[:])
                rec_tiles[(qg, hp)] = rec_bf
                tmp = sbn.tile([128, 512], BF, tag="tmp", name="tmp", bufs=3)
                # qg3 copies are emitted after every exp, so running them on
                # the (then-idle) ACT frees the pv PSUM slot without DVE
                # queueing delay -- the next PV chain is gated on this.
                if qg == 3:
                    nc.scalar.copy(tmp[:], pv[:])
                else:
                    nc.vector.tensor_copy(tmp[:], pv[:])
                tmp_tiles[(qg, hp)] = tmp

            def normalize_round(qg, hp):
                """rep-matmul + multiply -> attn_p[hp][qg] (both heads)."""
                rp = ps_misc.tile([128, 512], F32, tag="sm", name="rp")
                tmp = tmp_tiles.pop((qg, hp))
                rec_bf = rec_tiles.pop((qg, hp))
                for hh in range(2):
                    rows = slice(64 * hh, 64 * hh + 64)
                    nc.tensor.matmul(
                        rp[rows, :], ones_bf[0:1, 0:64], rec_bf[:, hh, :],
                        start=True, stop=True,
                    )
                nc.vector.tensor_mul(
                    attn_p[hp][qg][:], tmp[:], rp[:],
                )

            def outproj_chunk(qg):
                """partial out-projection rows for query group qg."""
                for tt4 in range(4):
                    tt = 4 * qg + tt4
                    o_sb = osb.tile([128, C], BF, tag="o", name="osb", bufs=3)
                    for cn in range(2):
                        op = ps_misc.tile(
                            [128, 512], F32, tag="sm", name="op"
                        )
                        for hp in range(2):
                            nc.tensor.matmul(
                                op[:],
                                attn_p[hp][qg][:, ts(tt4, 128)],
                                wout_sb[:, hp, ts(cn, 512)],
                                start=(hp == 0),
                                stop=(hp == 1),
                            )
                        # qg>=2 outproj copies are emitted after all exps
                        # (ACT queue is drained of them by then), so split
                        # the PSUM evacuation across ACT and DVE.
                        if qg >= 2 and cn == 0:
                            nc.scalar.copy(o_sb[:, ts(cn, 512)], op[:])
                        else:
                            nc.vector.tensor_copy(o_sb[:, ts(cn, 512)], op[:])
                    # out DMAs ride sync/gpsimd only -- the scalar queue's
                    # DIRECT2D descriptor gen would head-of-line-block the
                    # exp stream on the ACT sequencer. Last chunk: halves
                    # on both queues to shorten the final-DMA tail.
                    if qg == 3:
                        # both HWDGE queues (scalar is idle post-exps):
                        # avoids the gpsimd software-DGE's ~650ns/DMA
                        # serialized descriptor generation on the tail.
                        nc.sync.dma_start(
                            out=out[ts(tt, 128), 0:512], in_=o_sb[:, 0:512]
                        )
                        nc.scalar.dma_start(
                            out=out[ts(tt, 128), 512:1024],
                            in_=o_sb[:, 512:1024],
                        )
                    else:
                        oq = nc.sync if tt % 2 == 0 else nc.gpsimd
                        oq.dma_start(out=out[ts(tt, 128), :], in_=o_sb[:])

            # ---- flash-style startup: feed ACT as early as possible ----
            # Scores-critical work is emitted (= prioritized) strictly ahead
            # of the V projections, which are pure PE filler in the ACT-paced
            # slack of rounds 1-2; pv(0,0) directly follows V.
            # round 0 (qg0, hp0): k chunk0 + q chunk0(tt0); scores chase the
            # k tt-groups as they land.
            qk_group(1, 0, 0)
            qk_group(0, 0, 0)
            # bulk loads (4MB bf16 x for V + wv + wout) split gpsimd/sync;
            # they sit FIFO behind the critical x8 loads on each queue.
            nc.gpsimd.dma_start(out=wv_sb[:], in_=wv[:])
            xv_tiles = []
            for vtt in range(4):
                xv = osb.tile([128, 8, 512], BF, tag="xv", name="xv", bufs=2)
                q = nc.gpsimd if vtt < 2 else nc.sync
                q.dma_start(out=xv[:], in_=xt[:, vtt])
                xv_tiles.append(xv)
            nc.sync.dma_start(out=wout_sb[:], in_=wout[:])
            p00 = p_alloc(0, 0)
            st_seg(0, 0, range(0, 4), p00)
            # ---- PE warmup: emitted AFTER the round-0 critical chain so
            # the scheduler treats it as gap filler (keeps the HAM clock
            # gate fed during DMA waits) instead of queue-blocking the
            # first projections.
            warm_ps = ps_misc.tile([128, 512], F32, tag="sm", name="warm")
            for i in range(17):
                nc.tensor.matmul(
                    warm_ps[0:1, 0:128], warm_src[:, 0:1], warm_src[:, 0:128],
                    start=(i == 0), stop=(i == 16),
                )
            qk_group(1, 0, 1)
            st_seg(0, 0, range(4, 8), p00)
            qk_group(1, 0, 2)
            st_seg(0, 0, range(8, 12), p00)
            qk_group(1, 0, 3)
            st_seg(0, 0, range(12, 16), p00)

            # round 1 (qg0, hp1): k chunk1 + q chunk1(tt0)
            qk_group(1, 1, 0)
            qk_group(0, 1, 0)
            p01 = p_alloc(0, 1)
            st_seg(0, 1, range(0, 4), p01)
            qk_group(1, 1, 1)
            st_seg(0, 1, range(4, 8), p01)
            qk_group(1, 1, 2)
            st_seg(0, 1, range(8, 12), p01)
            qk_group(1, 1, 3)
            st_seg(0, 1, range(12, 16), p01)

            # round 2 (qg1, hp0) scores, then V in its ACT-slack
            qk_group(0, 0, 1)
            qk_group(0, 1, 1)
            st_part(1, 0)
            for tv in range(16):
                v_group(tv)

            # ---- pipelined main stream ----
            pv_part(0, 0)
            st_part(1, 1)
            pv_part(0, 1)
            qk_group(0, 0, 2)
            qk_group(0, 1, 2)
            st_part(2, 0)
            normalize_round(0, 0)
            pv_part(1, 0)
            st_part(2, 1)
            normalize_round(0, 1)
            outproj_chunk(0)
            pv_part(1, 1)
            qk_group(0, 0, 3)
            qk_group(0, 1, 3)
            st_part(3, 0)
            normalize_round(1, 0)
            pv_part(2, 0)
            st_part(3, 1)
            normalize_round(1, 1)
            outproj_chunk(1)
            pv_part(2, 1)
            normalize_round(2, 0)
            normalize_round(2, 1)
            outproj_chunk(2)
            pv_part(3, 0)
            normalize_round(3, 0)
            pv_part(3, 1)
            normalize_round(3, 1)
            outproj_chunk(3)

    nc.compile()
    return nc


def _get_nc():
    global _NC_CACHE
    if _NC_CACHE is None:
        _NC_CACHE = _build_nc()
    return _NC_CACHE


def kernel(x, w_qkv, b_qkv, w_out, b_out):
    global LAST_RESULT
    x = np.asarray(x, dtype=np.float32)
    w_qkv = np.asarray(w_qkv, dtype=np.float32)
    b_qkv = np.asarray(b_qkv, dtype=np.float32)
    w_out = np.asarray(w_out, dtype=np.float32)
    b_out = np.asarray(b_out, dtype=np.float32)

    bf = ml_dtypes.bfloat16

    f8 = ml_dtypes.float8_e4m3fn

    def blk_w(w):  # [1024, n] -> [128, 8, n] (p, ci, n) contiguous
        n = w.shape[1]
        return np.ascontiguousarray(
            w.reshape(8, 128, n).transpose(1, 0, 2)
        ).astype(bf)

    def blk_w8(w):  # [1024, n] -> [128, 4, 2, n] DoubleRow-packed fp8
        n = w.shape[1]
        return np.ascontiguousarray(
            w.reshape(4, 2, 128, n).transpose(2, 0, 1, 3).astype(f8)
        )

    in_maps = []
    for c in range(N_CORES):
        b, g = divmod(c, 4)
        cols = slice(CH * g, CH * (g + 1))
        bq = b_qkv[0 * C + CH * g : 0 * C + CH * (g + 1)]
        bk = b_qkv[1 * C + CH * g : 1 * C + CH * (g + 1)]
        # x^T token-blocked: [p, tt, ci, 512]
        xtb = np.ascontiguousarray(
            x[b].T.astype(bf).reshape(8, 128, 4, 512).transpose(1, 2, 0, 3)
        )
        # fp8 x^T DoubleRow-packed: [p, tt, jp, ko, 512]
        x8b = np.ascontiguousarray(
            x[b].T.reshape(4, 2, 128, 4, 512).transpose(2, 3, 0, 1, 4).astype(f8)
        )
        # wout row-blocked: [p, hp, 1024]
        wob = np.ascontiguousarray(
            w_out[CH * g : CH * (g + 1), :].reshape(2, 128, C).transpose(1, 0, 2)
        ).astype(bf)
        in_maps.append(
            {
                "xt": xtb,
                "xq8": x8b,
                "wq": blk_w8(w_qkv[:, 0 * C :][:, cols]),
                "wk": blk_w8(w_qkv[:, 1 * C :][:, cols]),
                "wv": blk_w(w_qkv[:, 2 * C :][:, cols]),
                "bqt": np.ascontiguousarray(bq.reshape(2, 128).T),
                "bkt": np.ascontiguousarray(bk.reshape(2, 128).T),
                "wout": wob,
            }
        )

    nc = _get_nc()
    LAST_RESULT = bass_utils.run_bass_kernel_spmd(
        nc, in_maps, core_ids=list(range(N_CORES))
    )

    full = np.zeros((B, T, C), dtype=np.float32)
    # bias folded once on the host: b_out plus the V-bias pushed through
    # w_out (normalized attention rows sum to 1, so bv contributes exactly
    # bv @ w_out to every token)
    full += b_out + b_qkv[2 * C : 3 * C] @ w_out
    for c in range(N_CORES):
        b = c // 4
        full[b] += LAST_RESULT.results[c]["out"].astype(np.float32)
    return full

# nonce 1

# nonce 2

# nonce 3

